# revision 55
# baseline (speedup 1.0000x reference)
"""Distributed Bass kernel for nn_AttentionLayer (B=2, S=2048, D=1024, H=16).

Sharding: tensor-parallel over heads. Core c owns heads {2c, 2c+1} (128 of the
1024 hidden dims). Each core:
  - projects q/k/v for its heads over all 4096 tokens (inputs fed pre-transposed
    as X^T so the contraction dim lands on SBUF partitions),
  - computes transposed scores scoreT[s,t] = k_h . q_h with the two heads packed
    into the PE array as K=64 row-tiles writing adjacent PSUM banks, exp on
    ScalarE over the combined [128,1024] tile, bias applied as a bf16 multiply
    with the host-precomputed exp(bias) (one big multiply per 4 key-tiles to
    amortize the DVE drain),
  - PV matmul with V (natural [s,dk] layout) as the stationary operand,
    augmented with a ones column so softmax denominators fall out of row 64,
  - normalizes LOCALLY: fast reciprocal of the denominator row straight from
    PSUM, replicated across the 64 dk partitions by a small broadcast DMA,
  - one AllToAll switches head-sharding -> token-sharding (minimal traffic:
    only the normalized attention outputs travel, no denominators), then each
    core runs the full output projection for its 512-token chunk.
Host side adds the output bias bo and reassembles (out, cache_k, cache_v).

Schedule: batch-0 projections run as a short lead-in (DMA spread over three
queues); batch-1 projections and V transposes are interleaved into the
batch-0 attention chunks so the PE stays busy (HAM-warm) and nothing
serializes behind the DMA.
"""

import sys

import numpy as np

for _p in ("/opt/trn_rl_repo",):
    if _p not in sys.path:
        sys.path.insert(0, _p)

import ml_dtypes

BF = ml_dtypes.bfloat16

B, S, D, H = 2, 2048, 1024, 16
DK = D // H            # 64
NCORES = 8
T = B * S              # 4096
OC = D // NCORES       # 128 hidden dims per core (2 heads)
CS = 512               # queries per chunk
NCH = T // CS          # 8 chunks
NST = S // 128         # 16 key tiles per batch

_CACHE = {}
DEBUG = False


def _build_nc():
    import concourse.bass as bass
    import concourse.mybir as mybir
    import concourse.tile as tile
    from concourse import bacc

    f32 = mybir.dt.float32
    bf16 = mybir.dt.bfloat16
    AF = mybir.ActivationFunctionType
    MUL = mybir.AluOpType.mult

    nc = bacc.Bacc(
        "TRN2",
        target_bir_lowering=False,
        debug=False,
        num_devices=NCORES,
    )

    # ---- kernel I/O ----
    # X pre-arranged host-side as [token-group, p, dc, t] so every slab load
    # is one contiguous run per partition (gather patterns run ~10x slower)
    xq = nc.dram_tensor("xq_g", [NCH, 128, 8, CS], bf16, kind="ExternalInput")
    xk = nc.dram_tensor("xk_g", [NCH, 128, 8, CS], bf16, kind="ExternalInput")
    xv = nc.dram_tensor("xv_g", [NCH, 128, 8, CS], bf16, kind="ExternalInput")
    # exp(bias) pre-arranged as [b, tcc, p, st, head-dup, q]
    ebias_p = nc.dram_tensor("ebias_p", [B, 4, 128, NST, 2, CS], bf16,
                             kind="ExternalInput")
    # weights pre-arranged [p, dc, m] (contiguous per partition)
    wq_t = nc.dram_tensor("wq_p", [128, 8, OC], bf16, kind="ExternalInput")
    wk_t = nc.dram_tensor("wk_p", [128, 8, OC], bf16, kind="ExternalInput")
    wv_t = nc.dram_tensor("wv_p", [128, 8, OC], bf16, kind="ExternalInput")
    # full WoT, laid out [d-within-slice, src-slice, od-group, od-within]
    wo_f = nc.dram_tensor("wo_f", [OC, NCORES, NCORES, OC], bf16,
                          kind="ExternalInput")
    b3_d = nc.dram_tensor("b3_c", [OC, 3], f32, kind="ExternalInput")
    ident_d = nc.dram_tensor("ident", [128, 128], bf16, kind="ExternalInput")

    kt_out = nc.dram_tensor("kt_out", [OC, T], bf16, kind="ExternalOutput")
    vt_out = nc.dram_tensor("vt_out", [OC, T], bf16, kind="ExternalOutput")
    out_t = nc.dram_tensor("out_t", [D, CS], bf16, kind="ExternalOutput")
    if DEBUG:
        dbg_mv = nc.dram_tensor("dbg_mv", [NCH, 128, CS], bf16,
                                kind="ExternalOutput")

    rg = [list(range(NCORES))]

    with tile.TileContext(nc) as tc:
        with tc.tile_pool(name="persist", bufs=1) as pp, \
             tc.tile_pool(name="dramp", bufs=1, space="DRAM") as dramp:
            ident = pp.tile([128, 128], bf16)
            nc.scalar.dma_start(ident[:], ident_d.ap())

            wq_sb = pp.tile([128, 8, OC], bf16)
            wk_sb = pp.tile([128, 8, OC], bf16)
            wv_sb = pp.tile([128, 8, OC], bf16)
            nc.scalar.dma_start(wk_sb[:], wk_t.ap())
            nc.scalar.dma_start(wv_sb[:], wv_t.ap())
            nc.scalar.dma_start(wq_sb[:], wq_t.ap())
            wo_sb = pp.tile([128, NCORES, NCORES, OC], bf16)
            b3_sb = pp.tile([OC, 3], f32)
            nc.scalar.dma_start(b3_sb[:], b3_d.ap())
            bq_sb = b3_sb[:, 0:1]
            bk_sb = b3_sb[:, 1:2]
            bv_sb = b3_sb[:, 2:3]

            # persistent activations
            qt_sb = pp.tile([OC, T], bf16)
            kt_sb = pp.tile([OC, T], bf16)
            # v natural [s, dk]; per 128-token tile the free axis is
            # [vA(64) | ones | vB(64) | ones].
            v_sb = pp.tile([128, 2 * NST, 130], bf16)
            nc.vector.memset(v_sb[:, :, 64:65], 1.0)
            nc.vector.memset(v_sb[:, :, 129:130], 1.0)

            # AllToAll buffers: normalized attention outputs, head-sharded ->
            # token-sharded. Minimal cross-core traffic (128 rows, no denom).
            a2a_in = dramp.tile([NCORES, OC, CS], bf16)
            a2a_out = dramp.tile([NCORES, OC, CS], bf16)

            with tc.tile_pool(name="work", bufs=2) as wk, \
                 tc.tile_pool(name="sc_ps", bufs=2, space="PSUM") as scps, \
                 tc.tile_pool(name="pv_ps", bufs=1, space="PSUM") as pvps, \
                 tc.tile_pool(name="ut_ps", bufs=1, space="PSUM") as utps:

                # ---------- helpers ----------
                def bias_prefetch(j):
                    # loaded duplicated for both heads so the per-quad bias
                    # multiply is one flat contiguous (2x-mode) tensor_tensor
                    b, tcc = divmod(j, 4)
                    bias_t = wk.tile([128, NST, 2, CS], bf16, tag="bias",
                                     bufs=2, name=f"bias{j}")
                    nc.gpsimd.dma_start(bias_t[:], ebias_p.ap()[b, tcc])
                    return bias_t

                _dmaq = [nc.sync, nc.scalar]
                _qi = [0]

                def proj_group(kind, b, tcg, lead_in):
                    """Project one [128-out, 512-token] tile of q/k/v.

                    kind: 'k' | 'q' | 'v'. tcg indexes 512-token groups within
                    batch b. Emits 1 DMA + 8 matmuls + evacuation (+ V
                    transposes)."""
                    x_d, w_sb, b_sb = {
                        "k": (xk, wk_sb, bk_sb),
                        "q": (xq, wq_sb, bq_sb),
                        "v": (xv, wv_sb, bv_sb),
                    }[kind]
                    t0 = b * S + tcg * CS
                    xsl = wk.tile([128, 8, CS], bf16, tag="xsl", bufs=3,
                                  name=f"xsl_{kind}{b}{tcg}")
                    if lead_in:
                        q = _dmaq[_qi[0] % 2]
                        _qi[0] += 1
                    else:
                        # keep sync free for the latency-critical rcb/a2a DMAs
                        q = nc.gpsimd
                    q.dma_start(xsl[:], x_d.ap()[b * 4 + tcg])
                    ps = utps.tile([128, CS], f32, tag="proj",
                                   name=f"ps_{kind}{b}{tcg}")
                    mms = [
                        lambda dc=dc: nc.tensor.matmul(
                            ps[:], w_sb[:, dc, :], xsl[:, dc, :],
                            start=(dc == 0), stop=(dc == 7))
                        for dc in range(8)
                    ]

                    def evac():
                        if kind == "k":
                            nc.vector.tensor_scalar_add(
                                kt_sb[:, t0:t0 + CS], ps[:], b_sb[:, 0:1])
                            nc.sync.dma_start(kt_out.ap()[:, t0:t0 + CS],
                                              kt_sb[:, t0:t0 + CS])
                            return []
                        if kind == "q":
                            nc.vector.tensor_scalar_add(
                                qt_sb[:, t0:t0 + CS], ps[:], b_sb[:, 0:1])
                            return []
                        vt = wk.tile([128, CS], bf16, tag="vt",
                                     name=f"vt{b}{tcg}")
                        nc.vector.tensor_scalar_add(vt[:], ps[:], b_sb[:, 0:1])
                        nc.sync.dma_start(vt_out.ap()[:, t0:t0 + CS], vt[:])
                        trs = []

                        def tr_one(i):
                            stg = b * NST + tcg * 4 + i
                            trp = utps.tile([128, 128], bf16, tag="proj",
                                            name=f"tr{b}{tcg}{i}")
                            nc.tensor.transpose(
                                trp[:], vt[:, i * 128:(i + 1) * 128], ident[:])
                            # GPSIMD cannot read PSUM; ACT is free during
                            # lead-in, DVE during attention.
                            if lead_in:
                                nc.scalar.copy(v_sb[:, stg, 0:64],
                                               trp[:, 0:64])
                                nc.scalar.copy(v_sb[:, stg, 65:129],
                                               trp[:, 64:128])
                            else:
                                nc.vector.tensor_copy(v_sb[:, stg, 0:64],
                                                      trp[:, 0:64])
                                nc.vector.tensor_copy(v_sb[:, stg, 65:129],
                                                      trp[:, 64:128])
                        for i in range(4):
                            trs.append(lambda i=i: tr_one(i))
                        return trs

                    return mms, evac

                def run_group(kind, b, tcg, lead_in=True):
                    mms, evac = proj_group(kind, b, tcg, lead_in)
                    for m in mms:
                        m()
                    for t in evac():
                        t()

                # ---------- lead-in: batch-0 projections ----------
                bias_tiles = {0: bias_prefetch(0)}
                for tcg in range(4):
                    run_group("k", 0, tcg)
                for tcg in range(4):
                    run_group("v", 0, tcg)
                run_group("q", 0, 0)
                bias_tiles[1] = bias_prefetch(1)

                # wo is only needed by the post-a2a tail; load it mid-run
                def load_wo():
                    nc.gpsimd.dma_start(wo_sb[:], wo_f.ap())

                # interleaved projection work, keyed by chunk index
                def deferred_groups(j):
                    steps = []
                    specs = {
                        0: [("q", 0, 1), ("q", 0, 2), ("q", 0, 3)],
                        1: [("k", 1, 0), ("k", 1, 1), ("k", 1, 2), ("k", 1, 3)],
                        2: [("v", 1, 0), ("v", 1, 1)],
                        3: [("v", 1, 2), ("v", 1, 3), ("q", 1, 0)],
                        4: [("q", 1, 1)],
                        5: [("q", 1, 2)],
                        6: [("q", 1, 3)],
                    }.get(j, [])
                    for kind, b, tcg in specs:
                        mms, evac = proj_group(kind, b, tcg, lead_in=False)
                        steps.extend(mms)
                        ev = evac
                        # evac returns transpose sub-steps (V); emit the
                        # evacuation itself as one item, each transpose as
                        # its own item so no single slot gets too heavy
                        trs_box = []

                        def run_evac(ev=ev, box=trs_box):
                            box.extend(ev())
                        steps.append(run_evac)

                        def run_tr(i, box=trs_box):
                            if i < len(box):
                                box[i]()
                        for i in range(4 if kind == "v" else 0):
                            steps.append(lambda i=i, box=trs_box: run_tr(i, box))
                    if j == 4:
                        steps.append(load_wo)
                    return steps

                # ---------- attention chunks ----------
                def norm_items(j, pvc):
                    """Deferred normalize + a2a staging for chunk j; runs
                    interleaved into chunk j+1 so the reciprocal/broadcast
                    latency stays off the critical path."""
                    items = []
                    rcp = wk.tile([65, 2, CS], f32, tag="rcp", name=f"rcp{j}")
                    rcb = wk.tile([64, 2, CS], f32, tag="rcb", name=f"rcb{j}")
                    mv = wk.tile([64, 2, CS], bf16, tag="mv", name=f"mv{j}")

                    def i0():
                        # one reciprocal covers both heads' denominators
                        nc.vector.reciprocal(rcp[64:65, :, :],
                                             pvc[64:65, :, :])
                        nc.sync.dma_start(
                            rcb[:, 0, :],
                            rcp[64:65, 0:1, :].broadcast_to((1, 64, CS)))
                        nc.sync.dma_start(
                            rcb[:, 1, :],
                            rcp[64:65, 1:2, :].broadcast_to((1, 64, CS)))

                    def i1():
                        nc.vector.tensor_tensor(mv[:, 0, :], pvc[0:64, 0, :],
                                                rcb[:, 0, :], MUL)
                        nc.sync.dma_start(a2a_in[j, 0:64, :], mv[:, 0, :])

                    def i2():
                        nc.vector.tensor_tensor(mv[:, 1, :],
                                                pvc[0:64, 1, :],
                                                rcb[:, 1, :], MUL)
                        nc.sync.dma_start(a2a_in[j, 64:128, :], mv[:, 1, :])
                        if DEBUG:
                            nc.scalar.dma_start(dbg_mv.ap()[j, 0:64, :],
                                                mv[:, 0, :])
                            nc.scalar.dma_start(dbg_mv.ap()[j, 64:128, :],
                                                mv[:, 1, :])
                    return [i0, i1, i2]

                pending = []
                for j in range(NCH):
                    b, tcc = divmod(j, 4)
                    tq = b * S + tcc * CS
                    bias_t = bias_tiles.pop(j)
                    if j + 2 < NCH:
                        bias_tiles[j + 2] = bias_prefetch(j + 2)
                    # previous chunk's normalize runs mid-chunk so its
                    # reciprocal never delays this chunk's first quad TT
                    dg = deferred_groups(j)
                    mid = max(1, len(dg) // 2)
                    extra = dg[:mid] + pending + dg[mid:]
                    ei = 0

                    pv2 = pvps.tile([65, 2, CS], f32, tag="pv", name=f"pv{j}")
                    pv_A = pv2[:, 0, :]
                    pv_B = pv2[:, 1, :]

                    def pv_mms(st, pt):
                        stg = b * NST + st
                        si = st % 4
                        nc.tensor.matmul(pv_A, v_sb[:, stg, 0:65],
                                         pt[:, si, 0, :],
                                         start=(st == 0),
                                         stop=(st == NST - 1))
                        nc.tensor.matmul(pv_B, v_sb[:, stg, 65:130],
                                         pt[:, si, 1, :],
                                         start=(st == 0),
                                         stop=(st == NST - 1))

                    # Software pipeline: scores+exp for quad q run while the
                    # PV matmuls of quad q-1 (whose bias-multiply finished
                    # during q's scores) interleave behind them, so a DVE
                    # lag never stalls the score/exp stream on the in-order
                    # PE queue. The bias multiply is ONE flat contiguous
                    # (2x-mode) TT per quad.
                    prev_pt = None
                    for q4 in range(NST // 4):
                        pr = wk.tile([128, 4, 2, CS], bf16, tag="pr", bufs=3,
                                     name=f"pr{j}_{q4}")
                        for si in range(4):
                            st = q4 * 4 + si
                            stg = b * NST + st
                            sc = scps.tile([128, 1024], f32, tag="sc",
                                           name=f"sc{j}_{st}")
                            nc.tensor.matmul(
                                sc[:, 0:CS],
                                kt_sb[0:64, stg * 128:(stg + 1) * 128],
                                qt_sb[0:64, tq:tq + CS],
                                start=True, stop=True)
                            nc.tensor.matmul(
                                sc[:, CS:2 * CS],
                                kt_sb[64:128, stg * 128:(stg + 1) * 128],
                                qt_sb[64:128, tq:tq + CS],
                                start=True, stop=True)
                            nc.scalar.activation(
                                pr[:, si, :, :].rearrange("p a b -> p (a b)"),
                                sc[:], AF.Exp)
                            if prev_pt is not None:
                                pv_mms(q4 * 4 + si - 4, prev_pt)
                            if ei < len(extra):
                                extra[ei]()
                                ei += 1
                            if ei < len(extra):
                                extra[ei]()
                                ei += 1
                        pt = wk.tile([128, 4, 2, CS], bf16, tag="pt", bufs=2,
                                     name=f"pt{j}_{q4}")
                        nc.vector.tensor_tensor(
                            pt[:].rearrange("p a b c -> p (a b c)"),
                            pr[:].rearrange("p a b c -> p (a b c)"),
                            bias_t[:, q4 * 4:(q4 + 1) * 4, :, :].rearrange(
                                "p a b c -> p (a b c)"),
                            MUL)
                        prev_pt = pt
                    for si in range(4):
                        pv_mms(12 + si, prev_pt)
                        if ei < len(extra):
                            extra[ei]()
                            ei += 1
                    while ei < len(extra):
                        extra[ei]()
                        ei += 1

                    # evacuate pv (incl. denominator row) to SBUF in one fast
                    # copy so the PSUM banks free for the next chunk; the
                    # normalize itself runs deferred inside chunk j+1.
                    pvc = wk.tile([65, 2, CS], f32, tag="pvc", name=f"pvc{j}")
                    nc.vector.tensor_copy(pvc[:], pv2[:])
                    pending = norm_items(j, pvc)
                for it in pending:
                    it()

                # ---------- AllToAll + output projection tail ----------
                nc.gpsimd.collective_compute(
                    "AllToAll", mybir.AluOpType.bypass, replica_groups=rg,
                    ins=[a2a_in[:].opt()], outs=[a2a_out[:].opt()])
                ao = wk.tile([128, NCORES, CS], bf16, tag="bias", name="ao")
                for s in range(NCORES):
                    (nc.sync if s % 2 == 0 else nc.scalar).dma_start(
                        ao[:, s, :], a2a_out[s])
                for g in range(NCORES):
                    pso = utps.tile([128, CS], f32,
                                    tag=("proj" if g % 2 == 0 else "po"),
                                    name=f"pso{g}")
                    for s in range(NCORES):
                        nc.tensor.matmul(pso[:], wo_sb[:, s, g, :],
                                         ao[:, s, :],
                                         start=(s == 0), stop=(s == 7))
                    ot = wk.tile([128, CS], bf16, tag="po", name=f"ot{g}")
                    nc.scalar.copy(ot[:], pso[:])
                    nc.sync.dma_start(out_t.ap()[g * OC:(g + 1) * OC, :],
                                      ot[:])

    return nc


def _get_nc():
    if "nc" not in _CACHE:
        nc = _build_nc()
        if not nc.is_finalized():
            nc.finalize()
        _CACHE["nc"] = nc
    return _CACHE["nc"]


def _prepare_in_maps(queries, keys, values, attn_bias, Wq, bq, Wk, bk, Wv, bv,
                     Wo, bo):
    f32 = np.float32

    def xprep(x):
        # [T, D] -> [g, p, dc, t] with [g, p, dc, t] = x.T[dc*128+p, g*512+t]
        xt = np.asarray(x, f32).reshape(T, D).T.astype(BF)  # [D, T]
        xr = xt.reshape(8, 128, NCH, CS)                    # [dc, p, g, t]
        return np.ascontiguousarray(xr.transpose(2, 1, 0, 3))

    xq_g = xprep(queries)
    xk_g = xprep(keys)
    xv_g = xprep(values)
    # exp(bias) -> [b, tcc, p, st, h(dup), q]
    eb = np.exp(np.transpose(np.asarray(attn_bias, f32)[:, 0],
                             (0, 2, 1))).astype(BF)          # [B, key, q]
    ebr = eb.reshape(B, NST, 128, 4, CS)                     # [b, st, p, tcc, q]
    ebias_p = np.ascontiguousarray(
        np.broadcast_to(ebr.transpose(0, 3, 2, 1, 4)[:, :, :, :, None, :],
                        (B, 4, 128, NST, 2, CS)))

    Wq = np.asarray(Wq, f32); Wk = np.asarray(Wk, f32)
    Wv = np.asarray(Wv, f32); Wo = np.asarray(Wo, f32)
    bq = np.asarray(bq, f32); bk = np.asarray(bk, f32)
    bv = np.asarray(bv, f32)

    scale = 1.0 / np.sqrt(np.float32(DK))
    # WoT [d, od] -> [d_in_slice, s, g, m] with d = s*128+d_in, od = g*128+m
    wo_f = np.ascontiguousarray(
        Wo.T.reshape(NCORES, OC, NCORES, OC).transpose(1, 0, 2, 3)).astype(BF)

    def wprep(w):
        # W[sl] rows -> stationary layout [p, dc, m]: [dc*128+p, m] of W.T
        return np.ascontiguousarray(
            w.T.reshape(8, 128, OC).transpose(1, 0, 2)).astype(BF)

    in_maps = []
    for c in range(NCORES):
        sl = slice(c * OC, (c + 1) * OC)
        b3 = np.stack([bq[sl] * scale, bk[sl], bv[sl]], axis=1)
        in_maps.append({
            "xq_g": xq_g, "xk_g": xk_g, "xv_g": xv_g, "ebias_p": ebias_p,
            "wq_p": wprep(Wq[sl] * scale),
            "wk_p": wprep(Wk[sl]),
            "wv_p": wprep(Wv[sl]),
            "wo_f": wo_f,
            "b3_c": np.ascontiguousarray(b3.astype(f32)),
            "ident": np.eye(128, dtype=np.float32).astype(BF),
        })
    return in_maps


def _run(in_maps, trace=False):
    from concourse.bass_utils import run_bass_kernel_spmd

    nc = _get_nc()
    return run_bass_kernel_spmd(nc, in_maps, core_ids=list(range(NCORES)),
                                trace=trace)


def _assemble(results, bo):
    out_full = np.empty((T, D), np.float32)
    k_full = np.empty((T, D), np.float32)
    v_full = np.empty((T, D), np.float32)
    for c in range(NCORES):
        r = results[c]
        k_full[:, c * OC:(c + 1) * OC] = np.asarray(r["kt_out"], np.float32).T
        v_full[:, c * OC:(c + 1) * OC] = np.asarray(r["vt_out"], np.float32).T
        b, tcc = divmod(c, 4)
        t0 = b * S + tcc * CS
        out_full[t0:t0 + CS, :] = np.asarray(r["out_t"], np.float32).T
    out_full += np.asarray(bo, np.float32)[None, :]
    return (out_full.reshape(B, S, D), k_full.reshape(B, S, D),
            v_full.reshape(B, S, D))


def kernel(**inputs):
    in_maps = _prepare_in_maps(**inputs)
    res = _run(in_maps, trace=False)
    return _assemble(res.results, inputs["bo"])


# revision 57
# speedup vs baseline: 1.1164x; 1.1164x over previous
"""Distributed Bass kernel for nn_AttentionLayer (B=2, S=2048, D=1024, H=16).

Sharding: tensor-parallel over heads. Core c owns heads {2c, 2c+1} (128 of the
1024 hidden dims). Each core:
  - projects q/k/v for its heads over all 4096 tokens (inputs fed pre-transposed
    as X^T so the contraction dim lands on SBUF partitions),
  - computes transposed scores scoreT[s,t] = k_h . q_h with the two heads packed
    into the PE array as K=64 row-tiles writing adjacent PSUM banks, exp on
    ScalarE over the combined [128,1024] tile, bias applied as a bf16 multiply
    with the host-precomputed exp(bias) (one big multiply per 4 key-tiles to
    amortize the DVE drain),
  - PV matmul with V (natural [s,dk] layout) as the stationary operand,
    augmented with a ones column so softmax denominators fall out of row 64,
  - normalizes LOCALLY: fast reciprocal of the denominator row straight from
    PSUM, replicated across the 64 dk partitions by a small broadcast DMA,
  - one AllToAll switches head-sharding -> token-sharding (minimal traffic:
    only the normalized attention outputs travel, no denominators), then each
    core runs the full output projection for its 512-token chunk.
Host side adds the output bias bo and reassembles (out, cache_k, cache_v).

Schedule: batch-0 projections run as a short lead-in (DMA spread over three
queues); batch-1 projections and V transposes are interleaved into the
batch-0 attention chunks so the PE stays busy (HAM-warm) and nothing
serializes behind the DMA.
"""

import sys

import numpy as np

for _p in ("/opt/trn_rl_repo",):
    if _p not in sys.path:
        sys.path.insert(0, _p)

import ml_dtypes

BF = ml_dtypes.bfloat16

B, S, D, H = 2, 2048, 1024, 16
DK = D // H            # 64
NCORES = 8
T = B * S              # 4096
OC = D // NCORES       # 128 hidden dims per core (2 heads)
CS = 512               # queries per chunk
NCH = T // CS          # 8 chunks
NST = S // 128         # 16 key tiles per batch

_CACHE = {}
DEBUG = False


def _build_nc():
    import concourse.bass as bass
    import concourse.mybir as mybir
    import concourse.tile as tile
    from concourse import bacc

    f32 = mybir.dt.float32
    bf16 = mybir.dt.bfloat16
    AF = mybir.ActivationFunctionType
    MUL = mybir.AluOpType.mult

    nc = bacc.Bacc(
        "TRN2",
        target_bir_lowering=False,
        debug=False,
        num_devices=NCORES,
    )

    # ---- kernel I/O ----
    # X pre-arranged host-side as [token-group, p, dc, t] so every slab load
    # is one contiguous run per partition (gather patterns run ~10x slower)
    xq = nc.dram_tensor("xq_g", [NCH, 128, 8, CS], bf16, kind="ExternalInput")
    xk = nc.dram_tensor("xk_g", [NCH, 128, 8, CS], bf16, kind="ExternalInput")
    xv = nc.dram_tensor("xv_g", [NCH, 128, 8, CS], bf16, kind="ExternalInput")
    # exp(bias) pre-arranged as [b, tcc, p, st, head-dup, q]
    ebias_p = nc.dram_tensor("ebias_p", [B, 4, 128, NST, 2, CS], bf16,
                             kind="ExternalInput")
    # weights pre-arranged [p, dc, m] (contiguous per partition)
    wq_t = nc.dram_tensor("wq_p", [128, 8, OC], bf16, kind="ExternalInput")
    wk_t = nc.dram_tensor("wk_p", [128, 8, OC], bf16, kind="ExternalInput")
    wv_t = nc.dram_tensor("wv_p", [128, 8, OC], bf16, kind="ExternalInput")
    # full WoT, laid out [d-within-slice, src-slice, od-group, od-within]
    wo_f = nc.dram_tensor("wo_f", [OC, NCORES, NCORES, OC], bf16,
                          kind="ExternalInput")
    b3_d = nc.dram_tensor("b3_c", [OC, 3], f32, kind="ExternalInput")
    ident_d = nc.dram_tensor("ident", [128, 128], bf16, kind="ExternalInput")

    kt_out = nc.dram_tensor("kt_out", [OC, T], bf16, kind="ExternalOutput")
    vt_out = nc.dram_tensor("vt_out", [OC, T], bf16, kind="ExternalOutput")
    out_t = nc.dram_tensor("out_t", [D, CS], bf16, kind="ExternalOutput")
    if DEBUG:
        dbg_mv = nc.dram_tensor("dbg_mv", [NCH, 128, CS], bf16,
                                kind="ExternalOutput")

    rg = [list(range(NCORES))]

    with tile.TileContext(nc) as tc:
        with tc.tile_pool(name="persist", bufs=1) as pp, \
             tc.tile_pool(name="dramp", bufs=1, space="DRAM") as dramp:
            ident = pp.tile([128, 128], bf16)
            nc.scalar.dma_start(ident[:], ident_d.ap())

            wq_sb = pp.tile([128, 8, OC], bf16)
            wk_sb = pp.tile([128, 8, OC], bf16)
            wv_sb = pp.tile([128, 8, OC], bf16)
            nc.scalar.dma_start(wk_sb[:], wk_t.ap())
            nc.scalar.dma_start(wv_sb[:], wv_t.ap())
            nc.scalar.dma_start(wq_sb[:], wq_t.ap())
            wo_sb = pp.tile([128, NCORES, NCORES, OC], bf16)
            b3_sb = pp.tile([OC, 3], f32)
            nc.scalar.dma_start(b3_sb[:], b3_d.ap())
            bq_sb = b3_sb[:, 0:1]
            bk_sb = b3_sb[:, 1:2]
            bv_sb = b3_sb[:, 2:3]

            # persistent activations
            qt_sb = pp.tile([OC, T], bf16)
            kt_sb = pp.tile([OC, T], bf16)
            # v natural [s, dk]; per 128-token tile the free axis is
            # [vA(64) | ones | vB(64) | ones].
            v_sb = pp.tile([128, 2 * NST, 130], bf16)
            nc.vector.memset(v_sb[:, :, 64:65], 1.0)
            nc.vector.memset(v_sb[:, :, 129:130], 1.0)

            # AllToAll buffers: normalized attention outputs, head-sharded ->
            # token-sharded. Minimal cross-core traffic (128 rows, no denom).
            a2a_in = dramp.tile([NCORES, OC, CS], bf16)
            a2a_out = dramp.tile([NCORES, OC, CS], bf16)

            with tc.tile_pool(name="work", bufs=2) as wk, \
                 tc.tile_pool(name="sc_ps", bufs=3, space="PSUM") as scps, \
                 tc.tile_pool(name="pv_ps", bufs=1, space="PSUM") as pvps:

                # ---------- helpers ----------
                def bias_prefetch(j):
                    # loaded duplicated for both heads so the per-quad bias
                    # multiply is one flat contiguous (2x-mode) tensor_tensor
                    b, tcc = divmod(j, 4)
                    bias_t = wk.tile([128, NST, 2, CS], bf16, tag="bias",
                                     bufs=2, name=f"bias{j}")
                    nc.gpsimd.dma_start(bias_t[:], ebias_p.ap()[b, tcc])
                    return bias_t

                _dmaq = [nc.sync, nc.scalar]
                _qi = [0]

                def proj_group(kind, b, tcg, lead_in):
                    """Project one [128-out, 512-token] tile of q/k/v.

                    kind: 'k' | 'q' | 'v'. tcg indexes 512-token groups within
                    batch b. Emits 1 DMA + 8 matmuls + evacuation (+ V
                    transposes)."""
                    x_d, w_sb, b_sb = {
                        "k": (xk, wk_sb, bk_sb),
                        "q": (xq, wq_sb, bq_sb),
                        "v": (xv, wv_sb, bv_sb),
                    }[kind]
                    t0 = b * S + tcg * CS
                    xsl = wk.tile([128, 8, CS], bf16, tag="xsl", bufs=3,
                                  name=f"xsl_{kind}{b}{tcg}")
                    if lead_in:
                        q = _dmaq[_qi[0] % 2]
                        _qi[0] += 1
                    else:
                        # keep sync free for the latency-critical rcb/a2a DMAs
                        q = nc.gpsimd
                    q.dma_start(xsl[:], x_d.ap()[b * 4 + tcg])
                    psf = scps.tile([128, 1024], f32, tag="sc",
                                    name=f"ps_{kind}{b}{tcg}")
                    ps = psf[:, 0:CS]
                    mms = [
                        lambda dc=dc: nc.tensor.matmul(
                            ps, w_sb[:, dc, :], xsl[:, dc, :],
                            start=(dc == 0), stop=(dc == 7))
                        for dc in range(8)
                    ]

                    def evac():
                        if kind == "k":
                            nc.vector.tensor_scalar_add(
                                kt_sb[:, t0:t0 + CS], ps, b_sb[:, 0:1])
                            nc.sync.dma_start(kt_out.ap()[:, t0:t0 + CS],
                                              kt_sb[:, t0:t0 + CS])
                            return []
                        if kind == "q":
                            nc.vector.tensor_scalar_add(
                                qt_sb[:, t0:t0 + CS], ps, b_sb[:, 0:1])
                            return []
                        vt = wk.tile([128, CS], bf16, tag="vt",
                                     name=f"vt{b}{tcg}")
                        nc.vector.tensor_scalar_add(vt[:], ps, b_sb[:, 0:1])
                        nc.sync.dma_start(vt_out.ap()[:, t0:t0 + CS], vt[:])
                        trs = []

                        def tr_one(i):
                            stg = b * NST + tcg * 4 + i
                            trp = scps.tile([128, 128], bf16, tag="sc",
                                            name=f"tr{b}{tcg}{i}")
                            nc.tensor.transpose(
                                trp[:], vt[:, i * 128:(i + 1) * 128], ident[:])
                            # GPSIMD cannot read PSUM; ACT is free during
                            # lead-in, DVE during attention.
                            if lead_in:
                                nc.scalar.copy(v_sb[:, stg, 0:64],
                                               trp[:, 0:64])
                                nc.scalar.copy(v_sb[:, stg, 65:129],
                                               trp[:, 64:128])
                            else:
                                nc.vector.tensor_copy(v_sb[:, stg, 0:64],
                                                      trp[:, 0:64])
                                nc.vector.tensor_copy(v_sb[:, stg, 65:129],
                                                      trp[:, 64:128])
                        for i in range(4):
                            trs.append(lambda i=i: tr_one(i))
                        return trs

                    return mms, evac

                def run_group(kind, b, tcg, lead_in=True):
                    mms, evac = proj_group(kind, b, tcg, lead_in)
                    for m in mms:
                        m()
                    for t in evac():
                        t()

                # ---------- lead-in: ALL projections ----------
                bias_tiles = {0: bias_prefetch(0)}
                for tcg in range(4):
                    run_group("k", 0, tcg)
                for tcg in range(4):
                    run_group("v", 0, tcg)
                run_group("q", 0, 0)
                bias_tiles[1] = bias_prefetch(1)
                for tcg in range(4):
                    run_group("k", 1, tcg)
                for tcg in range(4):
                    run_group("v", 1, tcg)
                for tcg in range(1, 4):
                    run_group("q", 0, tcg)
                for tcg in range(4):
                    run_group("q", 1, tcg)
                nc.gpsimd.dma_start(wo_sb[:], wo_f.ap())

                # wo is only needed by the post-a2a tail; load it mid-run
                def load_wo():
                    nc.gpsimd.dma_start(wo_sb[:], wo_f.ap())

                def deferred_groups(j):
                    return []

                # ---------- attention chunks ----------
                def norm_items(j, pvc):
                    """Deferred normalize + a2a staging for chunk j; runs
                    interleaved into chunk j+1 so the reciprocal/broadcast
                    latency stays off the critical path."""
                    items = []
                    rcp = wk.tile([65, 2, CS], f32, tag="rcp", name=f"rcp{j}")
                    rcb = wk.tile([64, 2, CS], f32, tag="rcb", name=f"rcb{j}")
                    mv = wk.tile([64, 2, CS], bf16, tag="mv", name=f"mv{j}")

                    def i0():
                        # one reciprocal covers both heads' denominators
                        nc.vector.reciprocal(rcp[64:65, :, :],
                                             pvc[64:65, :, :])
                        nc.sync.dma_start(
                            rcb[:, 0, :],
                            rcp[64:65, 0:1, :].broadcast_to((1, 64, CS)))
                        nc.sync.dma_start(
                            rcb[:, 1, :],
                            rcp[64:65, 1:2, :].broadcast_to((1, 64, CS)))

                    def i1():
                        nc.vector.tensor_tensor(mv[:, 0, :], pvc[0:64, 0, :],
                                                rcb[:, 0, :], MUL)
                        nc.sync.dma_start(a2a_in[j, 0:64, :], mv[:, 0, :])

                    def i2():
                        nc.vector.tensor_tensor(mv[:, 1, :],
                                                pvc[0:64, 1, :],
                                                rcb[:, 1, :], MUL)
                        nc.sync.dma_start(a2a_in[j, 64:128, :], mv[:, 1, :])
                        if DEBUG:
                            nc.scalar.dma_start(dbg_mv.ap()[j, 0:64, :],
                                                mv[:, 0, :])
                            nc.scalar.dma_start(dbg_mv.ap()[j, 64:128, :],
                                                mv[:, 1, :])
                    return [i0, i1, i2]

                pending = []
                for j in range(NCH):
                    b, tcc = divmod(j, 4)
                    tq = b * S + tcc * CS
                    bias_t = bias_tiles.pop(j)
                    if j + 2 < NCH:
                        bias_tiles[j + 2] = bias_prefetch(j + 2)
                    # previous chunk's normalize runs mid-chunk so its
                    # reciprocal never delays this chunk's first quad TT
                    dg = deferred_groups(j)
                    mid = max(1, len(dg) // 2)
                    extra = dg[:mid] + pending + dg[mid:]
                    ei = 0

                    pv2 = pvps.tile([65, 2, CS], f32, tag="pv", name=f"pv{j}")
                    pv_A = pv2[:, 0, :]
                    pv_B = pv2[:, 1, :]

                    def pv_mms(st, pt):
                        stg = b * NST + st
                        si = st % 4
                        nc.tensor.matmul(pv_A, v_sb[:, stg, 0:65],
                                         pt[:, si, 0, :],
                                         start=(st == 0),
                                         stop=(st == NST - 1))
                        nc.tensor.matmul(pv_B, v_sb[:, stg, 65:130],
                                         pt[:, si, 1, :],
                                         start=(st == 0),
                                         stop=(st == NST - 1))

                    # Software pipeline: scores+exp for quad q run while the
                    # PV matmuls of quad q-1 (whose bias-multiply finished
                    # during q's scores) interleave behind them, so a DVE
                    # lag never stalls the score/exp stream on the in-order
                    # PE queue. The bias multiply is ONE flat contiguous
                    # (2x-mode) TT per quad.
                    prev_pt = None
                    for q4 in range(NST // 4):
                        pr = wk.tile([128, 4, 2, CS], bf16, tag="pr", bufs=3,
                                     name=f"pr{j}_{q4}")
                        for si in range(4):
                            st = q4 * 4 + si
                            stg = b * NST + st
                            sc = scps.tile([128, 1024], f32, tag="sc",
                                           name=f"sc{j}_{st}")
                            nc.tensor.matmul(
                                sc[:, 0:CS],
                                kt_sb[0:64, stg * 128:(stg + 1) * 128],
                                qt_sb[0:64, tq:tq + CS],
                                start=True, stop=True)
                            nc.tensor.matmul(
                                sc[:, CS:2 * CS],
                                kt_sb[64:128, stg * 128:(stg + 1) * 128],
                                qt_sb[64:128, tq:tq + CS],
                                start=True, stop=True)
                            nc.scalar.activation(
                                pr[:, si, :, :].rearrange("p a b -> p (a b)"),
                                sc[:], AF.Exp)
                            if prev_pt is not None:
                                pv_mms(q4 * 4 + si - 4, prev_pt)
                            if ei < len(extra):
                                extra[ei]()
                                ei += 1
                            if ei < len(extra):
                                extra[ei]()
                                ei += 1
                        pt = wk.tile([128, 4, 2, CS], bf16, tag="pt", bufs=2,
                                     name=f"pt{j}_{q4}")
                        nc.vector.tensor_tensor(
                            pt[:].rearrange("p a b c -> p (a b c)"),
                            pr[:].rearrange("p a b c -> p (a b c)"),
                            bias_t[:, q4 * 4:(q4 + 1) * 4, :, :].rearrange(
                                "p a b c -> p (a b c)"),
                            MUL)
                        prev_pt = pt
                    for si in range(4):
                        pv_mms(12 + si, prev_pt)
                        if ei < len(extra):
                            extra[ei]()
                            ei += 1
                    while ei < len(extra):
                        extra[ei]()
                        ei += 1

                    # evacuate pv (incl. denominator row) to SBUF in one fast
                    # copy so the PSUM banks free for the next chunk; the
                    # normalize itself runs deferred inside chunk j+1.
                    pvc = wk.tile([65, 2, CS], f32, tag="pvc", name=f"pvc{j}")
                    nc.vector.tensor_copy(pvc[:], pv2[:])
                    pending = norm_items(j, pvc)
                for it in pending:
                    it()

                # ---------- AllToAll + output projection tail ----------
                nc.gpsimd.collective_compute(
                    "AllToAll", mybir.AluOpType.bypass, replica_groups=rg,
                    ins=[a2a_in[:].opt()], outs=[a2a_out[:].opt()])
                ao = wk.tile([128, NCORES, CS], bf16, tag="bias", name="ao")
                for s in range(NCORES):
                    (nc.sync if s % 2 == 0 else nc.scalar).dma_start(
                        ao[:, s, :], a2a_out[s])
                for g in range(NCORES):
                    psof = scps.tile([128, 1024], f32, tag="sc",
                                     name=f"pso{g}")
                    pso = psof[:, 0:CS]
                    for s in range(NCORES):
                        nc.tensor.matmul(pso, wo_sb[:, s, g, :],
                                         ao[:, s, :],
                                         start=(s == 0), stop=(s == 7))
                    ot = wk.tile([128, CS], bf16, tag="po", name=f"ot{g}")
                    nc.scalar.copy(ot[:], pso)
                    nc.sync.dma_start(out_t.ap()[g * OC:(g + 1) * OC, :],
                                      ot[:])

    return nc


def _get_nc():
    if "nc" not in _CACHE:
        nc = _build_nc()
        if not nc.is_finalized():
            nc.finalize()
        _CACHE["nc"] = nc
    return _CACHE["nc"]


def _prepare_in_maps(queries, keys, values, attn_bias, Wq, bq, Wk, bk, Wv, bv,
                     Wo, bo):
    f32 = np.float32

    def xprep(x):
        # [T, D] -> [g, p, dc, t] with [g, p, dc, t] = x.T[dc*128+p, g*512+t]
        xt = np.asarray(x, f32).reshape(T, D).T.astype(BF)  # [D, T]
        xr = xt.reshape(8, 128, NCH, CS)                    # [dc, p, g, t]
        return np.ascontiguousarray(xr.transpose(2, 1, 0, 3))

    xq_g = xprep(queries)
    xk_g = xprep(keys)
    xv_g = xprep(values)
    # exp(bias) -> [b, tcc, p, st, h(dup), q]
    eb = np.exp(np.transpose(np.asarray(attn_bias, f32)[:, 0],
                             (0, 2, 1))).astype(BF)          # [B, key, q]
    ebr = eb.reshape(B, NST, 128, 4, CS)                     # [b, st, p, tcc, q]
    ebias_p = np.ascontiguousarray(
        np.broadcast_to(ebr.transpose(0, 3, 2, 1, 4)[:, :, :, :, None, :],
                        (B, 4, 128, NST, 2, CS)))

    Wq = np.asarray(Wq, f32); Wk = np.asarray(Wk, f32)
    Wv = np.asarray(Wv, f32); Wo = np.asarray(Wo, f32)
    bq = np.asarray(bq, f32); bk = np.asarray(bk, f32)
    bv = np.asarray(bv, f32)

    scale = 1.0 / np.sqrt(np.float32(DK))
    # WoT [d, od] -> [d_in_slice, s, g, m] with d = s*128+d_in, od = g*128+m
    wo_f = np.ascontiguousarray(
        Wo.T.reshape(NCORES, OC, NCORES, OC).transpose(1, 0, 2, 3)).astype(BF)

    def wprep(w):
        # W[sl] rows -> stationary layout [p, dc, m]: [dc*128+p, m] of W.T
        return np.ascontiguousarray(
            w.T.reshape(8, 128, OC).transpose(1, 0, 2)).astype(BF)

    in_maps = []
    for c in range(NCORES):
        sl = slice(c * OC, (c + 1) * OC)
        b3 = np.stack([bq[sl] * scale, bk[sl], bv[sl]], axis=1)
        in_maps.append({
            "xq_g": xq_g, "xk_g": xk_g, "xv_g": xv_g, "ebias_p": ebias_p,
            "wq_p": wprep(Wq[sl] * scale),
            "wk_p": wprep(Wk[sl]),
            "wv_p": wprep(Wv[sl]),
            "wo_f": wo_f,
            "b3_c": np.ascontiguousarray(b3.astype(f32)),
            "ident": np.eye(128, dtype=np.float32).astype(BF),
        })
    return in_maps


def _run(in_maps, trace=False):
    from concourse.bass_utils import run_bass_kernel_spmd

    nc = _get_nc()
    return run_bass_kernel_spmd(nc, in_maps, core_ids=list(range(NCORES)),
                                trace=trace)


def _assemble(results, bo):
    out_full = np.empty((T, D), np.float32)
    k_full = np.empty((T, D), np.float32)
    v_full = np.empty((T, D), np.float32)
    for c in range(NCORES):
        r = results[c]
        k_full[:, c * OC:(c + 1) * OC] = np.asarray(r["kt_out"], np.float32).T
        v_full[:, c * OC:(c + 1) * OC] = np.asarray(r["vt_out"], np.float32).T
        b, tcc = divmod(c, 4)
        t0 = b * S + tcc * CS
        out_full[t0:t0 + CS, :] = np.asarray(r["out_t"], np.float32).T
    out_full += np.asarray(bo, np.float32)[None, :]
    return (out_full.reshape(B, S, D), k_full.reshape(B, S, D),
            v_full.reshape(B, S, D))


def kernel(**inputs):
    in_maps = _prepare_in_maps(**inputs)
    res = _run(in_maps, trace=False)
    return _assemble(res.results, inputs["bo"])


# revision 58
# speedup vs baseline: 1.1309x; 1.0130x over previous
"""Distributed Bass kernel for nn_AttentionLayer (B=2, S=2048, D=1024, H=16).

Sharding: tensor-parallel over heads. Core c owns heads {2c, 2c+1} (128 of the
1024 hidden dims). Each core:
  - projects q/k/v for its heads over all 4096 tokens (inputs fed pre-transposed
    as X^T so the contraction dim lands on SBUF partitions),
  - computes transposed scores scoreT[s,t] = k_h . q_h with the two heads packed
    into the PE array as K=64 row-tiles writing adjacent PSUM banks, adds the
    shared attn bias (b=0: identity-stationary matmul into PSUM on the PE;
    b=1: tensor_tensor add on the otherwise-idle DVE), exp on ScalarE over the
    combined [128,1024] tile,
  - PV matmul with V (natural [s,dk] layout) as the stationary operand,
    augmented with a ones column so softmax denominators fall out of row 64,
  - AllToAll switches head-sharding -> token-sharding (each core ends up with
    all heads for its 512-token slice), normalizes, and applies the output
    projection for its token slice.
Host side reassembles (out, cache_k, cache_v) from per-core slices.
"""

import sys

import numpy as np

for _p in ("/opt/trn_rl_repo",):
    if _p not in sys.path:
        sys.path.insert(0, _p)

import ml_dtypes

BF = ml_dtypes.bfloat16

B, S, D, H = 2, 2048, 1024, 16
DK = D // H            # 64
NCORES = 8
T = B * S              # 4096
OC = D // NCORES       # 128 hidden dims per core (2 heads)
TSL = T // NCORES      # 512 token slice per core after AllToAll

_CACHE = {}


def _build_nc():
    import concourse.bass as bass
    import concourse.mybir as mybir
    import concourse.tile as tile
    from concourse import bacc

    f32 = mybir.dt.float32
    bf16 = mybir.dt.bfloat16
    AF = mybir.ActivationFunctionType

    nc = bacc.Bacc(
        "TRN2",
        target_bir_lowering=False,
        debug=False,
        num_devices=NCORES,
    )

    # ---- kernel I/O ----
    xq = nc.dram_tensor("xq_t", [D, T], bf16, kind="ExternalInput")
    xk = nc.dram_tensor("xk_t", [D, T], bf16, kind="ExternalInput")
    xv = nc.dram_tensor("xv_t", [D, T], bf16, kind="ExternalInput")
    ebias_t = nc.dram_tensor("ebias_t", [B, S, S], bf16, kind="ExternalInput")
    wq_t = nc.dram_tensor("wq_t", [D, OC], bf16, kind="ExternalInput")
    wk_t = nc.dram_tensor("wk_t", [D, OC], bf16, kind="ExternalInput")
    wv_t = nc.dram_tensor("wv_t", [D, OC], bf16, kind="ExternalInput")
    wo_t = nc.dram_tensor("wo_t", [D, D], bf16, kind="ExternalInput")
    bq_d = nc.dram_tensor("bq_c", [OC, 1], f32, kind="ExternalInput")
    bk_d = nc.dram_tensor("bk_c", [OC, 1], f32, kind="ExternalInput")
    bv_d = nc.dram_tensor("bv_c", [OC, 1], f32, kind="ExternalInput")
    bo_d = nc.dram_tensor("bo_f", [D, 1], f32, kind="ExternalInput")
    sel_d = nc.dram_tensor("sel", [H, D], bf16, kind="ExternalInput")
    ident_d = nc.dram_tensor("ident", [128, 128], bf16, kind="ExternalInput")

    kt_out = nc.dram_tensor("kt_out", [OC, T], f32, kind="ExternalOutput")
    vt_out = nc.dram_tensor("vt_out", [OC, T], f32, kind="ExternalOutput")
    out_t = nc.dram_tensor("out_t", [D, TSL], f32, kind="ExternalOutput")

    rg = [list(range(NCORES))]

    with tile.TileContext(nc) as tc:
        with tc.tile_pool(name="persist", bufs=1) as pp, \
             tc.tile_pool(name="dramp", bufs=1, space="DRAM") as dramp:
            ident = pp.tile([128, 128], bf16)
            nc.sync.dma_start(ident[:], ident_d.ap())

            wq_sb = pp.tile([128, 8, OC], bf16)
            wk_sb = pp.tile([128, 8, OC], bf16)
            wv_sb = pp.tile([128, 8, OC], bf16)
            nc.sync.dma_start(wq_sb[:], wq_t.ap().rearrange("(c p) m -> p c m", p=128))
            nc.sync.dma_start(wk_sb[:], wk_t.ap().rearrange("(c p) m -> p c m", p=128))
            nc.sync.dma_start(wv_sb[:], wv_t.ap().rearrange("(c p) m -> p c m", p=128))
            wo_sb = pp.tile([128, 8, D], bf16)
            nc.sync.dma_start(wo_sb[:], wo_t.ap().rearrange("(c p) m -> p c m", p=128))
            sel_sb = pp.tile([H, D], bf16)
            nc.sync.dma_start(sel_sb[:], sel_d.ap())
            bq_sb = pp.tile([OC, 1], f32)
            bk_sb = pp.tile([OC, 1], f32)
            nc.sync.dma_start(bq_sb[:], bq_d.ap())
            nc.sync.dma_start(bk_sb[:], bk_d.ap())
            bv_sb = pp.tile([OC, 1], f32)
            nc.sync.dma_start(bv_sb[:], bv_d.ap())
            bo_sb = pp.tile([128, 8], f32)
            nc.sync.dma_start(bo_sb[:], bo_d.ap().rearrange("(c p) o -> p (c o)", p=128))
            # persistent activations
            qt_sb = pp.tile([OC, T], bf16)       # qT for this core's 2 heads
            kt_sb = pp.tile([OC, T], bf16)       # kT
            vt_sb = pp.tile([OC, T], bf16)       # vT (transposed to v_sb below)
            # v in natural [s, dk] layout; per 128-token chunk the free axis is
            # [vA(64) | ones | vB(64) | ones] so head slices 0:65 / 65:130 give
            # the ones-augmented PV stationary directly.
            v_sb = pp.tile([128, T // 128, 130], bf16)
            nc.vector.memset(v_sb[:, :, 64:65], 1.0)
            nc.vector.memset(v_sb[:, :, 129:130], 1.0)

            # collective bounce buffers (DRAM)
            a2a_in = dramp.tile([NCORES, OC + 4, TSL], bf16)
            a2a_out = dramp.tile([NCORES, OC + 4, TSL], bf16)

            # ================= phase 1: projections =================
            # d-chunk outer; one 1MB X^T chunk DMA feeds 8 token-chunk matmuls
            # (k, v) or accumulates into 8 PSUM banks (k/q: one per t-chunk;
            # v: 4 128-token tiles packed per bank).
            with tc.tile_pool(name="proj_ps", bufs=8, space="PSUM") as prps, \
                 tc.tile_pool(name="proj_sb", bufs=3) as prsb, \
                 tc.tile_pool(name="proj_ev", bufs=3) as prev:
                # --- K projection (kT layout: [o, t]) ---
                ps_k = [prps.tile([128, 512], f32, tag="ps", name=f"ps_k{i}") for i in range(8)]
                for dc in range(8):
                    xt = prsb.tile([128, T], bf16, tag="xch")
                    xr = xk.ap()[dc * 128:(dc + 1) * 128, :]
                    nc.sync.dma_start(xt[:, 0:1536], xr[:, 0:1536])
                    nc.scalar.dma_start(xt[:, 1536:3072], xr[:, 1536:3072])
                    nc.gpsimd.dma_start(xt[:, 3072:T], xr[:, 3072:T])
                    for tcg in range(8):
                        nc.tensor.matmul(ps_k[tcg][:], wk_sb[:, dc, :],
                                         xt[:, tcg * 512:(tcg + 1) * 512],
                                         start=(dc == 0), stop=(dc == 7))
                for tcg in range(8):
                    kf = prev.tile([128, 512], f32, tag="kf32")
                    nc.vector.tensor_scalar_add(kf[:], ps_k[tcg][:], bk_sb[:, 0:1])
                    nc.sync.dma_start(kt_out.ap()[:, tcg * 512:(tcg + 1) * 512], kf[:])
                    nc.vector.tensor_copy(kt_sb[:, tcg * 512:(tcg + 1) * 512], kf[:])

                # --- V projection (vT layout like K; transposed afterwards) ---
                ps_v = [prps.tile([128, 512], f32, tag="ps", name=f"ps_v{i}") for i in range(8)]
                for dc in range(8):
                    xt = prsb.tile([128, T], bf16, tag="xch")
                    xr = xv.ap()[dc * 128:(dc + 1) * 128, :]
                    nc.sync.dma_start(xt[:, 0:1536], xr[:, 0:1536])
                    nc.scalar.dma_start(xt[:, 1536:3072], xr[:, 1536:3072])
                    nc.gpsimd.dma_start(xt[:, 3072:T], xr[:, 3072:T])
                    for tcg in range(8):
                        nc.tensor.matmul(ps_v[tcg][:], wv_sb[:, dc, :],
                                         xt[:, tcg * 512:(tcg + 1) * 512],
                                         start=(dc == 0), stop=(dc == 7))
                for tcg in range(8):
                    vf = prev.tile([128, 512], f32, tag="vf32")
                    nc.vector.tensor_scalar_add(vf[:], ps_v[tcg][:], bv_sb[:, 0:1])
                    nc.sync.dma_start(vt_out.ap()[:, tcg * 512:(tcg + 1) * 512], vf[:])
                    nc.vector.tensor_copy(vt_sb[:, tcg * 512:(tcg + 1) * 512], vf[:])

                # --- Q projection (qT layout; 1/sqrt(dk) folded in host-side) ---
                ps_q = [prps.tile([128, 512], f32, tag="ps", name=f"ps_q{i}") for i in range(8)]
                for dc in range(8):
                    xt = prsb.tile([128, T], bf16, tag="xch")
                    xr = xq.ap()[dc * 128:(dc + 1) * 128, :]
                    nc.sync.dma_start(xt[:, 0:1536], xr[:, 0:1536])
                    nc.scalar.dma_start(xt[:, 1536:3072], xr[:, 1536:3072])
                    nc.gpsimd.dma_start(xt[:, 3072:T], xr[:, 3072:T])
                    for tcg in range(8):
                        nc.tensor.matmul(ps_q[tcg][:], wq_sb[:, dc, :],
                                         xt[:, tcg * 512:(tcg + 1) * 512],
                                         start=(dc == 0), stop=(dc == 7))
                for tcg in range(8):
                    nc.vector.tensor_scalar_add(
                        qt_sb[:, tcg * 512:(tcg + 1) * 512], ps_q[tcg][:],
                        bq_sb[:, 0:1])

            # --- transpose vT -> v natural [s, dk] with ones-augmented layout ---
            with tc.tile_pool(name="tr_ps", bufs=4, space="PSUM") as trps:
                for stg in range(T // 128):
                    ps_t = trps.tile([128, 128], bf16, tag="ps_t")
                    nc.tensor.transpose(
                        ps_t[:], vt_sb[:, stg * 128:(stg + 1) * 128], ident[:])
                    nc.vector.tensor_copy(v_sb[:, stg, 0:64], ps_t[:, 0:64])
                    nc.vector.tensor_copy(v_sb[:, stg, 65:129], ps_t[:, 64:128])

            # ================= phase 2: attention =================
            # scores for both heads land in one [128,1024] (2-bank) PSUM tile:
            # head A in [:,0:512], head B in [:,512:1024] (row-packed K=64
            # matmuls into adjacent banks run concurrently), one FD=1024 exp.
            with tc.tile_pool(name="sc_ps", bufs=3, space="PSUM") as scps, \
                 tc.tile_pool(name="pv_ps", bufs=1, space="PSUM") as pvps, \
                 tc.tile_pool(name="at_sb", bufs=3) as atsb:
                for b in range(B):
                    for tcc in range(4):
                        j = b * 4 + tcc            # destination core / t slice
                        tg = b * S + tcc * 512     # global token offset
                        pv_A = pvps.tile([65, 512], f32, tag="pv_A")
                        pv_B = pvps.tile([65, 512], f32, tag="pv_B")
                        for st in range(S // 128):
                            stg = b * (S // 128) + st
                            bias_tile = atsb.tile([128, 512], bf16, tag="bias")
                            nc.gpsimd.dma_start(
                                bias_tile[:],
                                ebias_t.ap()[b, st * 128:(st + 1) * 128,
                                             tcc * 512:(tcc + 1) * 512])
                            ps = scps.tile([128, 1024], f32, tag="ps_sc")
                            # transposed scores, two heads as K=64 row tiles
                            nc.tensor.matmul(
                                ps[:, 0:512],
                                kt_sb[0:64, stg * 128:(stg + 1) * 128],
                                qt_sb[0:64, tg:tg + 512],
                                start=True, stop=True)
                            nc.tensor.matmul(
                                ps[:, 512:1024],
                                kt_sb[64:128, stg * 128:(stg + 1) * 128],
                                qt_sb[64:128, tg:tg + 512],
                                start=True, stop=True)
                            # p = exp(score) * exp(bias): exp on ACT, then a
                            # cheap bf16 2x-mode multiply on DVE
                            pr = atsb.tile([128, 1024], bf16, tag="pr")
                            nc.scalar.activation(pr[:], ps[:], AF.Exp)
                            pt = atsb.tile([128, 1024], bf16, tag="pt")
                            nc.vector.tensor_tensor(
                                pt[:, 0:512], pr[:, 0:512], bias_tile[:],
                                mybir.AluOpType.mult)
                            nc.vector.tensor_tensor(
                                pt[:, 512:1024], pr[:, 512:1024], bias_tile[:],
                                mybir.AluOpType.mult)
                            nc.tensor.matmul(pv_A[:], v_sb[:, stg, 0:65],
                                             pt[:, 0:512],
                                             start=(st == 0), stop=(st == 15))
                            nc.tensor.matmul(pv_B[:], v_sb[:, stg, 65:130],
                                             pt[:, 512:1024],
                                             start=(st == 0), stop=(st == 15))
                        at_A = atsb.tile([64, 512], bf16, tag="at_A")
                        at_B = atsb.tile([64, 512], bf16, tag="at_B")
                        nc.vector.tensor_copy(at_A[:], pv_A[0:64, :])
                        nc.vector.tensor_copy(at_B[:], pv_B[0:64, :])
                        nc.sync.dma_start(a2a_in[j, 0:64, :], at_A[:])
                        nc.sync.dma_start(a2a_in[j, 64:128, :], at_B[:])
                        dn = atsb.tile([65, 2, 512], f32, tag="dn")
                        nc.vector.tensor_copy(dn[64:65, 0, :],
                                              pv_A[64:65, :])
                        nc.vector.tensor_copy(dn[64:65, 1, :],
                                              pv_B[64:65, :])
                        nc.sync.dma_start(a2a_in[j, OC:OC + 4, :],
                                          dn[64:65, :, :].bitcast(bf16))

            # ================= phase 3: collectives =================
            nc.gpsimd.collective_compute(
                "AllToAll", mybir.AluOpType.bypass, replica_groups=rg,
                ins=[a2a_in[:].opt()], outs=[a2a_out[:].opt()])

            # ================= phase 4: normalize + output proj =================
            with tc.tile_pool(name="po_ps", bufs=8, space="PSUM") as pops, \
                 tc.tile_pool(name="po_sb", bufs=2) as posb:
                attn_rv = pp.tile([128, 8, TSL], bf16)
                nc.sync.dma_start(attn_rv[:],
                                  a2a_out[:, 0:OC, :].rearrange("c p f -> p c f"))
                den_rv = posb.tile([H, TSL], f32, tag="den_rv")
                den_sp = den_rv[:].rearrange("(c u) f -> c u f", u=2)
                for u in range(2):
                    nc.sync.dma_start(
                        den_sp[:, u, :],
                        a2a_out[:, OC + 2 * u:OC + 2 * u + 2, :].bitcast(
                            f32).rearrange("c t f -> c (t f)"))
                rcp = posb.tile([H, TSL], f32, tag="rcp")
                nc.vector.reciprocal(rcp[:], den_rv[:])
                rcpb = posb.tile([H, TSL], bf16, tag="rcpb")
                nc.vector.tensor_copy(rcpb[:], rcp[:])
                attn_n = pp.tile([128, 8, TSL], bf16)
                rgs = []
                for oc in range(8):
                    ps_rg = pops.tile([128, 512], f32, tag="ps_rg",
                                      name=f"ps_rg{oc}")
                    nc.tensor.matmul(
                        ps_rg[:], sel_sb[:, oc * 128:(oc + 1) * 128], rcpb[:],
                        start=True, stop=True)
                    rgs.append(ps_rg)
                for oc in range(8):
                    nc.vector.tensor_tensor(attn_n[:, oc, :], attn_rv[:, oc, :],
                                            rgs[oc][:], mybir.AluOpType.mult)
                for do in range(8):
                    ps_o = pops.tile([128, 512], f32, tag="ps_rg", name=f"ps_o{do}")
                    for oc in range(8):
                        nc.tensor.matmul(
                            ps_o[:], wo_sb[:, oc, do * 128:(do + 1) * 128],
                            attn_n[:, oc, :],
                            start=(oc == 0), stop=(oc == 7))
                    of = posb.tile([128, 512], f32, tag="of")
                    nc.vector.tensor_scalar_add(of[:], ps_o[:], bo_sb[:, do:do + 1])
                    nc.sync.dma_start(out_t.ap()[do * 128:(do + 1) * 128, :], of[:])

    return nc


def _get_nc():
    if "nc" not in _CACHE:
        nc = _build_nc()
        if not nc.is_finalized():
            nc.finalize()
        _CACHE["nc"] = nc
    return _CACHE["nc"]


def _prepare_in_maps(queries, keys, values, attn_bias, Wq, bq, Wk, bk, Wv, bv,
                     Wo, bo):
    f32 = np.float32
    xq_t = np.ascontiguousarray(
        np.asarray(queries, f32).reshape(T, D).T).astype(BF)
    xk_t = np.ascontiguousarray(
        np.asarray(keys, f32).reshape(T, D).T).astype(BF)
    xv_t = np.ascontiguousarray(
        np.asarray(values, f32).reshape(T, D).T).astype(BF)
    ebias_t = np.ascontiguousarray(
        np.exp(np.transpose(np.asarray(attn_bias, f32)[:, 0], (0, 2, 1)))).astype(BF)

    Wq = np.asarray(Wq, f32); Wk = np.asarray(Wk, f32)
    Wv = np.asarray(Wv, f32); Wo = np.asarray(Wo, f32)
    bq = np.asarray(bq, f32); bk = np.asarray(bk, f32)
    bv = np.asarray(bv, f32); bo = np.asarray(bo, f32)

    scale = 1.0 / np.sqrt(np.float32(DK))
    wo_t = np.ascontiguousarray(Wo.T).astype(BF)
    bo_f = np.ascontiguousarray(bo.reshape(D, 1))
    sel = np.zeros((H, D), np.float32)
    for o in range(D):
        sel[o // DK, o] = 1.0

    in_maps = []
    for c in range(NCORES):
        sl = slice(c * OC, (c + 1) * OC)
        in_maps.append({
            "xq_t": xq_t, "xk_t": xk_t, "xv_t": xv_t, "ebias_t": ebias_t,
            "wq_t": np.ascontiguousarray((Wq[sl] * scale).T).astype(BF),
            "wk_t": np.ascontiguousarray(Wk[sl].T).astype(BF),
            "wv_t": np.ascontiguousarray(Wv[sl].T).astype(BF),
            "wo_t": wo_t,
            "bq_c": np.ascontiguousarray((bq[sl] * scale).reshape(OC, 1)),
            "bk_c": np.ascontiguousarray(bk[sl].reshape(OC, 1)),
            "bv_c": np.ascontiguousarray(bv[sl].reshape(OC, 1)),
            "bo_f": bo_f,
            "sel": sel.astype(BF),
            "ident": np.eye(128, dtype=np.float32).astype(BF),
        })
    return in_maps


def _run(in_maps, trace=False):
    from concourse.bass_utils import run_bass_kernel_spmd

    nc = _get_nc()
    return run_bass_kernel_spmd(nc, in_maps, core_ids=list(range(NCORES)),
                                trace=trace)


def _assemble(results):
    out_full = np.empty((T, D), np.float32)
    k_full = np.empty((T, D), np.float32)
    v_full = np.empty((T, D), np.float32)
    for c in range(NCORES):
        r = results[c]
        k_full[:, c * OC:(c + 1) * OC] = r["kt_out"].T
        v_full[:, c * OC:(c + 1) * OC] = r["vt_out"].T
        out_full[c * TSL:(c + 1) * TSL, :] = r["out_t"].T
    return (out_full.reshape(B, S, D), k_full.reshape(B, S, D),
            v_full.reshape(B, S, D))


def kernel(**inputs):
    in_maps = _prepare_in_maps(**inputs)
    res = _run(in_maps, trace=False)
    return _assemble(res.results)



# revision 59
# speedup vs baseline: 1.1895x; 1.0518x over previous
"""Distributed Bass kernel for nn_AttentionLayer (B=2, S=2048, D=1024, H=16).

Sharding: tensor-parallel over heads. Core c owns heads {2c, 2c+1} (128 of the
1024 hidden dims). Each core:
  - projects q/k/v for its heads over all 4096 tokens (inputs fed pre-transposed
    as X^T so the contraction dim lands on SBUF partitions),
  - computes transposed scores scoreT[s,t] = k_h . q_h with the two heads packed
    into the PE array as K=64 row-tiles writing adjacent PSUM banks, adds the
    shared attn bias (b=0: identity-stationary matmul into PSUM on the PE;
    b=1: tensor_tensor add on the otherwise-idle DVE), exp on ScalarE over the
    combined [128,1024] tile,
  - PV matmul with V (natural [s,dk] layout) as the stationary operand,
    augmented with a ones column so softmax denominators fall out of row 64,
  - AllToAll switches head-sharding -> token-sharding (each core ends up with
    all heads for its 512-token slice), normalizes, and applies the output
    projection for its token slice.
Host side reassembles (out, cache_k, cache_v) from per-core slices.
"""

import sys

import numpy as np

for _p in ("/opt/trn_rl_repo",):
    if _p not in sys.path:
        sys.path.insert(0, _p)

import ml_dtypes

BF = ml_dtypes.bfloat16

B, S, D, H = 2, 2048, 1024, 16
DK = D // H            # 64
NCORES = 8
T = B * S              # 4096
OC = D // NCORES       # 128 hidden dims per core (2 heads)
TSL = T // NCORES      # 512 token slice per core after AllToAll

_CACHE = {}


def _build_nc():
    import concourse.bass as bass
    import concourse.mybir as mybir
    import concourse.tile as tile
    from concourse import bacc

    f32 = mybir.dt.float32
    bf16 = mybir.dt.bfloat16
    AF = mybir.ActivationFunctionType

    nc = bacc.Bacc(
        "TRN2",
        target_bir_lowering=False,
        debug=False,
        num_devices=NCORES,
    )

    # ---- kernel I/O ----
    xq = nc.dram_tensor("xq_t", [D, T], bf16, kind="ExternalInput")
    xk = nc.dram_tensor("xk_t", [D, T], bf16, kind="ExternalInput")
    xv = nc.dram_tensor("xv_t", [D, T], bf16, kind="ExternalInput")
    ebias_t = nc.dram_tensor("ebias_t", [B, S, S], bf16, kind="ExternalInput")
    wq_t = nc.dram_tensor("wq_p", [128, 8, OC], bf16, kind="ExternalInput")
    wk_t = nc.dram_tensor("wk_p", [128, 8, OC], bf16, kind="ExternalInput")
    wv_t = nc.dram_tensor("wv_p", [128, 8, OC], bf16, kind="ExternalInput")
    wo_t = nc.dram_tensor("wo_p", [128, 8, D], bf16, kind="ExternalInput")
    bq_d = nc.dram_tensor("bq_c", [OC, 1], f32, kind="ExternalInput")
    bk_d = nc.dram_tensor("bk_c", [OC, 1], f32, kind="ExternalInput")
    bv_d = nc.dram_tensor("bv_c", [OC, 1], f32, kind="ExternalInput")
    bo_d = nc.dram_tensor("bo_p", [128, 8], f32, kind="ExternalInput")
    sel_d = nc.dram_tensor("sel", [H, D], bf16, kind="ExternalInput")
    ident_d = nc.dram_tensor("ident", [128, 128], bf16, kind="ExternalInput")

    kt_out = nc.dram_tensor("kt_out", [OC, T], bf16, kind="ExternalOutput")
    vt_out = nc.dram_tensor("vt_out", [OC, T], bf16, kind="ExternalOutput")
    out_t = nc.dram_tensor("out_t", [D, TSL], f32, kind="ExternalOutput")

    rg = [list(range(NCORES))]

    with tile.TileContext(nc) as tc:
        with tc.tile_pool(name="persist", bufs=1) as pp, \
             tc.tile_pool(name="dramp", bufs=1, space="DRAM") as dramp:
            ident = pp.tile([128, 128], bf16)
            nc.sync.dma_start(ident[:], ident_d.ap())

            wq_sb = pp.tile([128, 8, OC], bf16)
            wk_sb = pp.tile([128, 8, OC], bf16)
            wv_sb = pp.tile([128, 8, OC], bf16)
            nc.sync.dma_start(wk_sb[:], wk_t.ap())
            nc.sync.dma_start(wv_sb[:], wv_t.ap())
            nc.sync.dma_start(wq_sb[:], wq_t.ap())
            wo_sb = pp.tile([128, 8, D], bf16)
            nc.scalar.dma_start(wo_sb[:], wo_t.ap())
            sel_sb = pp.tile([H, D], bf16)
            nc.sync.dma_start(sel_sb[:], sel_d.ap())
            bq_sb = pp.tile([OC, 1], f32)
            bk_sb = pp.tile([OC, 1], f32)
            nc.sync.dma_start(bq_sb[:], bq_d.ap())
            nc.sync.dma_start(bk_sb[:], bk_d.ap())
            bv_sb = pp.tile([OC, 1], f32)
            nc.sync.dma_start(bv_sb[:], bv_d.ap())
            bo_sb = pp.tile([128, 8], f32)
            nc.scalar.dma_start(bo_sb[:], bo_d.ap())
            # persistent activations
            qt_sb = pp.tile([OC, T], bf16)       # qT for this core's 2 heads
            kt_sb = pp.tile([OC, T], bf16)       # kT
            vt_sb = pp.tile([OC, T], bf16)       # vT (transposed to v_sb below)
            # v in natural [s, dk] layout; per 128-token chunk the free axis is
            # [vA(64) | ones | vB(64) | ones] so head slices 0:65 / 65:130 give
            # the ones-augmented PV stationary directly.
            v_sb = pp.tile([128, T // 128, 130], bf16)
            nc.vector.memset(v_sb[:, :, 64:65], 1.0)
            nc.vector.memset(v_sb[:, :, 129:130], 1.0)

            # collective bounce buffers (DRAM)
            a2a_in = dramp.tile([NCORES, OC + 4, TSL], bf16)
            a2a_out = dramp.tile([NCORES, OC + 4, TSL], bf16)

            # ================= phase 1: projections =================
            # d-chunk outer; one 1MB X^T chunk DMA feeds 8 token-chunk matmuls
            # (k, v) or accumulates into 8 PSUM banks (k/q: one per t-chunk;
            # v: 4 128-token tiles packed per bank).
            with tc.tile_pool(name="proj_ps", bufs=8, space="PSUM") as prps, \
                 tc.tile_pool(name="proj_sb", bufs=3) as prsb, \
                 tc.tile_pool(name="proj_ev", bufs=3) as prev:
                # --- K projection (kT layout: [o, t]) ---
                ps_k = [prps.tile([128, 512], f32, tag="ps", name=f"ps_k{i}") for i in range(8)]
                for dc in range(8):
                    xt = prsb.tile([128, T], bf16, tag="xch")
                    xr = xk.ap()[dc * 128:(dc + 1) * 128, :]
                    nc.sync.dma_start(xt[:, 0:1536], xr[:, 0:1536])
                    nc.scalar.dma_start(xt[:, 1536:3072], xr[:, 1536:3072])
                    nc.gpsimd.dma_start(xt[:, 3072:T], xr[:, 3072:T])
                    for tcg in range(8):
                        nc.tensor.matmul(ps_k[tcg][:], wk_sb[:, dc, :],
                                         xt[:, tcg * 512:(tcg + 1) * 512],
                                         start=(dc == 0), stop=(dc == 7))
                for tcg in range(8):
                    nc.vector.tensor_scalar_add(
                        kt_sb[:, tcg * 512:(tcg + 1) * 512], ps_k[tcg][:],
                        bk_sb[:, 0:1])
                    nc.sync.dma_start(kt_out.ap()[:, tcg * 512:(tcg + 1) * 512],
                                      kt_sb[:, tcg * 512:(tcg + 1) * 512])

                # --- V projection (vT layout like K; transposed afterwards) ---
                ps_v = [prps.tile([128, 512], f32, tag="ps", name=f"ps_v{i}") for i in range(8)]
                for dc in range(8):
                    xt = prsb.tile([128, T], bf16, tag="xch")
                    xr = xv.ap()[dc * 128:(dc + 1) * 128, :]
                    nc.sync.dma_start(xt[:, 0:1536], xr[:, 0:1536])
                    nc.scalar.dma_start(xt[:, 1536:3072], xr[:, 1536:3072])
                    nc.gpsimd.dma_start(xt[:, 3072:T], xr[:, 3072:T])
                    for tcg in range(8):
                        nc.tensor.matmul(ps_v[tcg][:], wv_sb[:, dc, :],
                                         xt[:, tcg * 512:(tcg + 1) * 512],
                                         start=(dc == 0), stop=(dc == 7))
                for tcg in range(8):
                    nc.vector.tensor_scalar_add(
                        vt_sb[:, tcg * 512:(tcg + 1) * 512], ps_v[tcg][:],
                        bv_sb[:, 0:1])
                    nc.sync.dma_start(vt_out.ap()[:, tcg * 512:(tcg + 1) * 512],
                                      vt_sb[:, tcg * 512:(tcg + 1) * 512])

                # --- Q projection (qT layout; 1/sqrt(dk) folded in host-side) ---
                ps_q = [prps.tile([128, 512], f32, tag="ps", name=f"ps_q{i}") for i in range(8)]
                for dc in range(8):
                    xt = prsb.tile([128, T], bf16, tag="xch")
                    xr = xq.ap()[dc * 128:(dc + 1) * 128, :]
                    nc.sync.dma_start(xt[:, 0:1536], xr[:, 0:1536])
                    nc.scalar.dma_start(xt[:, 1536:3072], xr[:, 1536:3072])
                    nc.gpsimd.dma_start(xt[:, 3072:T], xr[:, 3072:T])
                    for tcg in range(8):
                        nc.tensor.matmul(ps_q[tcg][:], wq_sb[:, dc, :],
                                         xt[:, tcg * 512:(tcg + 1) * 512],
                                         start=(dc == 0), stop=(dc == 7))
                for tcg in range(8):
                    nc.vector.tensor_scalar_add(
                        qt_sb[:, tcg * 512:(tcg + 1) * 512], ps_q[tcg][:],
                        bq_sb[:, 0:1])

            # --- transpose vT -> v natural [s, dk] with ones-augmented layout ---
            with tc.tile_pool(name="tr_ps", bufs=4, space="PSUM") as trps:
                for stg in range(T // 128):
                    ps_t = trps.tile([128, 128], bf16, tag="ps_t")
                    nc.tensor.transpose(
                        ps_t[:], vt_sb[:, stg * 128:(stg + 1) * 128], ident[:])
                    nc.vector.tensor_copy(v_sb[:, stg, 0:64], ps_t[:, 0:64])
                    nc.vector.tensor_copy(v_sb[:, stg, 65:129], ps_t[:, 64:128])

            # ================= phase 2: attention =================
            # scores for both heads land in one [128,1024] (2-bank) PSUM tile:
            # head A in [:,0:512], head B in [:,512:1024] (row-packed K=64
            # matmuls into adjacent banks run concurrently), one FD=1024 exp.
            with tc.tile_pool(name="sc_ps", bufs=3, space="PSUM") as scps, \
                 tc.tile_pool(name="pv_ps", bufs=1, space="PSUM") as pvps, \
                 tc.tile_pool(name="at_sb", bufs=3) as atsb:
                for b in range(B):
                    for tcc in range(4):
                        j = b * 4 + tcc            # destination core / t slice
                        tg = b * S + tcc * 512     # global token offset
                        pv_A = pvps.tile([65, 512], f32, tag="pv_A")
                        pv_B = pvps.tile([65, 512], f32, tag="pv_B")
                        for st in range(S // 128):
                            stg = b * (S // 128) + st
                            bias_tile = atsb.tile([128, 512], bf16, tag="bias")
                            nc.gpsimd.dma_start(
                                bias_tile[:],
                                ebias_t.ap()[b, st * 128:(st + 1) * 128,
                                             tcc * 512:(tcc + 1) * 512])
                            ps = scps.tile([128, 1024], f32, tag="ps_sc")
                            # transposed scores, two heads as K=64 row tiles
                            nc.tensor.matmul(
                                ps[:, 0:512],
                                kt_sb[0:64, stg * 128:(stg + 1) * 128],
                                qt_sb[0:64, tg:tg + 512],
                                start=True, stop=True)
                            nc.tensor.matmul(
                                ps[:, 512:1024],
                                kt_sb[64:128, stg * 128:(stg + 1) * 128],
                                qt_sb[64:128, tg:tg + 512],
                                start=True, stop=True)
                            # p = exp(score) * exp(bias): exp on ACT, then a
                            # cheap bf16 2x-mode multiply on DVE
                            pr = atsb.tile([128, 1024], bf16, tag="pr")
                            nc.scalar.activation(pr[:], ps[:], AF.Exp)
                            pt = atsb.tile([128, 1024], bf16, tag="pt")
                            nc.vector.tensor_tensor(
                                pt[:, 0:512], pr[:, 0:512], bias_tile[:],
                                mybir.AluOpType.mult)
                            nc.vector.tensor_tensor(
                                pt[:, 512:1024], pr[:, 512:1024], bias_tile[:],
                                mybir.AluOpType.mult)
                            nc.tensor.matmul(pv_A[:], v_sb[:, stg, 0:65],
                                             pt[:, 0:512],
                                             start=(st == 0), stop=(st == 15))
                            nc.tensor.matmul(pv_B[:], v_sb[:, stg, 65:130],
                                             pt[:, 512:1024],
                                             start=(st == 0), stop=(st == 15))
                        at_A = atsb.tile([64, 512], bf16, tag="at_A")
                        at_B = atsb.tile([64, 512], bf16, tag="at_B")
                        nc.vector.tensor_copy(at_A[:], pv_A[0:64, :])
                        nc.vector.tensor_copy(at_B[:], pv_B[0:64, :])
                        nc.sync.dma_start(a2a_in[j, 0:64, :], at_A[:])
                        nc.sync.dma_start(a2a_in[j, 64:128, :], at_B[:])
                        dn = atsb.tile([65, 2, 512], f32, tag="dn")
                        nc.vector.tensor_copy(dn[64:65, 0, :],
                                              pv_A[64:65, :])
                        nc.vector.tensor_copy(dn[64:65, 1, :],
                                              pv_B[64:65, :])
                        nc.sync.dma_start(a2a_in[j, OC:OC + 4, :],
                                          dn[64:65, :, :].bitcast(bf16))

            # ================= phase 3: collectives =================
            nc.gpsimd.collective_compute(
                "AllToAll", mybir.AluOpType.bypass, replica_groups=rg,
                ins=[a2a_in[:].opt()], outs=[a2a_out[:].opt()])

            # ================= phase 4: normalize + output proj =================
            with tc.tile_pool(name="po_ps", bufs=8, space="PSUM") as pops, \
                 tc.tile_pool(name="po_sb", bufs=2) as posb:
                attn_rv = pp.tile([128, 8, TSL], bf16)
                nc.sync.dma_start(attn_rv[:],
                                  a2a_out[:, 0:OC, :].rearrange("c p f -> p c f"))
                den_rv = posb.tile([H, TSL], f32, tag="den_rv")
                den_sp = den_rv[:].rearrange("(c u) f -> c u f", u=2)
                for u in range(2):
                    nc.sync.dma_start(
                        den_sp[:, u, :],
                        a2a_out[:, OC + 2 * u:OC + 2 * u + 2, :].bitcast(
                            f32).rearrange("c t f -> c (t f)"))
                rcp = posb.tile([H, TSL], f32, tag="rcp")
                nc.vector.reciprocal(rcp[:], den_rv[:])
                rcpb = posb.tile([H, TSL], bf16, tag="rcpb")
                nc.vector.tensor_copy(rcpb[:], rcp[:])
                attn_n = pp.tile([128, 8, TSL], bf16)
                rgs = []
                for oc in range(8):
                    ps_rg = pops.tile([128, 512], f32, tag="ps_rg",
                                      name=f"ps_rg{oc}")
                    nc.tensor.matmul(
                        ps_rg[:], sel_sb[:, oc * 128:(oc + 1) * 128], rcpb[:],
                        start=True, stop=True)
                    rgs.append(ps_rg)
                for oc in range(8):
                    nc.vector.tensor_tensor(attn_n[:, oc, :], attn_rv[:, oc, :],
                                            rgs[oc][:], mybir.AluOpType.mult)
                for do in range(8):
                    ps_o = pops.tile([128, 512], f32, tag="ps_rg", name=f"ps_o{do}")
                    for oc in range(8):
                        nc.tensor.matmul(
                            ps_o[:], wo_sb[:, oc, do * 128:(do + 1) * 128],
                            attn_n[:, oc, :],
                            start=(oc == 0), stop=(oc == 7))
                    of = posb.tile([128, 512], f32, tag="of")
                    nc.vector.tensor_scalar_add(of[:], ps_o[:], bo_sb[:, do:do + 1])
                    nc.sync.dma_start(out_t.ap()[do * 128:(do + 1) * 128, :], of[:])

    return nc


def _get_nc():
    if "nc" not in _CACHE:
        nc = _build_nc()
        if not nc.is_finalized():
            nc.finalize()
        _CACHE["nc"] = nc
    return _CACHE["nc"]


def _prepare_in_maps(queries, keys, values, attn_bias, Wq, bq, Wk, bk, Wv, bv,
                     Wo, bo):
    f32 = np.float32
    xq_t = np.ascontiguousarray(
        np.asarray(queries, f32).reshape(T, D).T).astype(BF)
    xk_t = np.ascontiguousarray(
        np.asarray(keys, f32).reshape(T, D).T).astype(BF)
    xv_t = np.ascontiguousarray(
        np.asarray(values, f32).reshape(T, D).T).astype(BF)
    ebias_t = np.ascontiguousarray(
        np.exp(np.transpose(np.asarray(attn_bias, f32)[:, 0], (0, 2, 1)))).astype(BF)

    Wq = np.asarray(Wq, f32); Wk = np.asarray(Wk, f32)
    Wv = np.asarray(Wv, f32); Wo = np.asarray(Wo, f32)
    bq = np.asarray(bq, f32); bk = np.asarray(bk, f32)
    bv = np.asarray(bv, f32); bo = np.asarray(bo, f32)

    scale = 1.0 / np.sqrt(np.float32(DK))

    def wprep(w, ncols):
        return np.ascontiguousarray(
            w.T.reshape(8, 128, ncols).transpose(1, 0, 2)).astype(BF)

    wo_p = wprep(Wo, D)
    bo_p = np.ascontiguousarray(bo.reshape(8, 128).T.astype(np.float32))
    sel = np.zeros((H, D), np.float32)
    for o in range(D):
        sel[o // DK, o] = 1.0

    in_maps = []
    for c in range(NCORES):
        sl = slice(c * OC, (c + 1) * OC)
        in_maps.append({
            "xq_t": xq_t, "xk_t": xk_t, "xv_t": xv_t, "ebias_t": ebias_t,
            "wq_p": wprep(Wq[sl] * scale, OC),
            "wk_p": wprep(Wk[sl], OC),
            "wv_p": wprep(Wv[sl], OC),
            "wo_p": wo_p,
            "bq_c": np.ascontiguousarray((bq[sl] * scale).reshape(OC, 1)),
            "bk_c": np.ascontiguousarray(bk[sl].reshape(OC, 1)),
            "bv_c": np.ascontiguousarray(bv[sl].reshape(OC, 1)),
            "bo_p": bo_p,
            "sel": sel.astype(BF),
            "ident": np.eye(128, dtype=np.float32).astype(BF),
        })
    return in_maps


def _run(in_maps, trace=False):
    from concourse.bass_utils import run_bass_kernel_spmd

    nc = _get_nc()
    return run_bass_kernel_spmd(nc, in_maps, core_ids=list(range(NCORES)),
                                trace=trace)


def _assemble(results):
    out_full = np.empty((T, D), np.float32)
    k_full = np.empty((T, D), np.float32)
    v_full = np.empty((T, D), np.float32)
    for c in range(NCORES):
        r = results[c]
        k_full[:, c * OC:(c + 1) * OC] = np.asarray(r["kt_out"],
                                                    np.float32).T
        v_full[:, c * OC:(c + 1) * OC] = np.asarray(r["vt_out"],
                                                    np.float32).T
        out_full[c * TSL:(c + 1) * TSL, :] = r["out_t"].T
    return (out_full.reshape(B, S, D), k_full.reshape(B, S, D),
            v_full.reshape(B, S, D))


def kernel(**inputs):
    in_maps = _prepare_in_maps(**inputs)
    res = _run(in_maps, trace=False)
    return _assemble(res.results)

